# revision 3
# baseline (speedup 1.0000x reference)
"""Trainium2 Bass kernel for nn_BERTCharting (pairwise-concat MLP).

Reference computation (per batch b):
    p = repr_w[b] @ W1[:H]        # [N, HID]
    q = repr_w[b] @ W1[H:]        # [N, HID]
    h[i,j,:] = relu(p[j] + q[i] + b1)
    out[i,j,:] = h[i,j] @ W2 + b2

Sharding: data-parallel over batch B=8 across the 8 NeuronCores (one batch
element per core). No collectives.

v3 design (per core):
  - inputs host-packed so every DMA descriptor is a contiguous >=512B
    per-partition row at full HWDGE rate; load order reprT -> W1[d=0]
    -> W1[d=1] -> W1[d=2] -> W2/b1 so the first GEMM's d=0 chain (and
    then h-gen) starts ~4us in, under the remaining input streams.
  - first GEMM on PE -> pT[d] bf16 / qbT[d]=qT+b1 fp32 (d-sequential;
    p/q accumulation chains interleaved so the PE pipeline overlaps).
  - h-gen: per-i ops [128 dpart, 128 j] (per-partition scalar q[d,i]
    caps the free dim at N=128; DVE dual-op tensor_scalar ~163 ns/op
    pipelined, ACT activation(Relu,bias) ~292 ns/op). 13/48 of ops go
    to ACT, balancing both engines including ACT's eviction load.
  - main GEMM: 8 mega-tiles of 16 i's, PSUM po[100, 2048] (4 banks,
    double-buffered), d-OUTER matmul order (stationary W2[d] held for 4
    MMs into different 512-col slices -> no PSUM output dep between
    neighbours). Last tile g-OUTER with per-group eviction so the
    pipeline drain is ~3us instead of ~9.
  - eviction: ACT copy [100, 2048] fp32 PSUM->SBUF (~1.9us/tile), then
    one DMA with 100 contiguous 8KiB descriptors into outT[l, i, j]
    (host transposes to [i,j,l]).
  - steady state is DVE/ACT-bound on the 6.3M-element relu(p+q) stream
    (both ~100% busy; per-i op floor).
"""

import os
import sys

for _p in ("/opt/trn_rl_repo",):
    if _p not in sys.path and os.path.isdir(_p):
        sys.path.insert(0, _p)

import numpy as np
import ml_dtypes

import concourse.mybir as mybir
from concourse import bacc, bass
from concourse.tile import TileContext
from concourse.bass_utils import run_bass_kernel_spmd


def _ensure_ntff_hook():
    """Provide antenv.axon_hooks (NTFF profile get/set) if the image lacks it,
    and install the ctypes-based profile hook against libaxon_pjrt.so so that
    run_bass_kernel_spmd(trace=True) can capture hardware profiles."""
    try:
        from antenv.axon_hooks import get_axon_ntff_profile_hook  # noqa: F401
        return
    except ImportError:
        pass
    import contextlib
    import ctypes
    import types

    mod = types.ModuleType("antenv.axon_hooks")
    holder = {"hook": None}
    mod.set_axon_ntff_profile_hook = lambda h: holder.__setitem__("hook", h)
    mod.get_axon_ntff_profile_hook = lambda: holder["hook"]
    sys.modules["antenv.axon_hooks"] = mod
    try:
        import antenv
        antenv.axon_hooks = mod
    except ImportError:
        pass

    so_path = "/opt/axon/libaxon_pjrt.so"
    if not os.path.exists(so_path):
        return
    lib = ctypes.CDLL(so_path)
    if not hasattr(lib, "axon_start_nrt_profile"):
        return
    lib.axon_start_nrt_profile.argtypes = [
        ctypes.POINTER(ctypes.c_int64),
        ctypes.c_size_t,
    ]
    lib.axon_start_nrt_profile.restype = ctypes.c_int64
    lib.axon_stop_nrt_profile.argtypes = [ctypes.c_char_p]
    lib.axon_stop_nrt_profile.restype = ctypes.c_int64

    @contextlib.contextmanager
    def _hook(output_dir, device_ids):
        import jax

        jax.devices()
        if device_ids:
            ids = (ctypes.c_int64 * len(device_ids))(*device_ids)
            rc = lib.axon_start_nrt_profile(ids, len(device_ids))
        else:
            rc = lib.axon_start_nrt_profile(None, 0)
        if rc != 0:
            raise RuntimeError(f"axon_start_nrt_profile rc={rc}")
        try:
            yield
        finally:
            n = lib.axon_stop_nrt_profile(str(output_dir).encode())
            print(f"ntff profile: {n} file(s) written to {output_dir}",
                  file=sys.stderr)

    mod.set_axon_ntff_profile_hook(_hook)


_ensure_ntff_hook()

B, N, H = 8, 128, 768
HID, L = 384, 100
NCORES = 8
KT = H // 128          # 6 contraction tiles for the first GEMM
DT = HID // 128        # 3 d-tiles
GROUP = 4              # i's per 512-col psum slice
TILE_G = 4             # groups per psum mega-tile (4 banks)
TILE_I = GROUP * TILE_G        # 16 i's per mega-tile
NTILES = N // TILE_I           # 8 mega-tiles

F32 = mybir.dt.float32
BF16 = mybir.dt.bfloat16

# Of the 48 h-gen ops per mega-tile, this many go to ACT (rest DVE);
# balances DVE (~163ns/op) vs ACT (~292ns/op + 15us eviction load).
ACT_OPS_PER_TILE = 13

# Stash of the last run's BassKernelResults (test harness reads exec_time_ns).
LAST_RESULT = None


def _build_program():
    nc = bacc.Bacc(None, target_bir_lowering=False)

    # Host-packed layouts: per-partition rows contiguous in DRAM.
    reprP = nc.declare_dram_parameter("reprP", [128, KT * N], BF16,
                                      isOutput=False)
    # w1p[d][p, (half,k)*128+c] = W1[half*H + k*128 + p, d*128 + c]
    w1p = nc.declare_dram_parameter("w1p", [DT, 128, 2 * KT * 128], BF16,
                                    isOutput=False)
    # w2p[p, d*L+l] = W2[d*128+p, l]
    w2p = nc.declare_dram_parameter("w2p", [128, DT * L], BF16,
                                    isOutput=False)
    b1c = nc.declare_dram_parameter("b1c", [128, DT], F32, isOutput=False)
    # Output l-major: outT[l, i, j]; host transposes back to [i, j, l].
    outT = nc.declare_dram_parameter("outT", [L, N, N], F32, isOutput=True)

    add = mybir.AluOpType.add
    maxop = mybir.AluOpType.max

    with TileContext(nc) as tc:
        with tc.tile_pool(name="const", bufs=1) as cpool:
            # ---- input loads: one full-rate DMA per chunk, d-ordered ------
            reprT_big = cpool.tile([128, KT, N], BF16, tag="reprTb",
                                   name="reprTb")
            nc.sync.dma_start(
                out=reprT_big,
                in_=reprP[:].rearrange("p (k n) -> p k n", k=KT),
            )
            reprT_sb = [reprT_big[:, k, :] for k in range(KT)]
            w1_sb = []
            for d in range(DT):
                w1d = cpool.tile([128, 2 * KT, 128], BF16, tag=f"w1d{d}",
                                 name=f"w1d{d}")
                nc.sync.dma_start(
                    out=w1d,
                    in_=w1p[d, :, :].rearrange("p (q c) -> p q c", q=2 * KT),
                )
                w1_sb.append(w1d)
            w2_big = cpool.tile([128, DT, L], BF16, tag="w2b", name="w2b")
            nc.sync.dma_start(
                out=w2_big,
                in_=w2p[:].rearrange("p (d l) -> p d l", d=DT),
            )
            w2_sb = [w2_big[:, d, :] for d in range(DT)]
            b1_sb = cpool.tile([128, DT], F32, tag="b1c", name="b1sb")
            nc.sync.dma_start(out=b1_sb, in_=b1c[:, :])

            # ---- first GEMMs: pT, qbT (d-sequential, p/q interleaved) -----
            pT, qbT = [], []
            with tc.tile_pool(name="ps1", bufs=1, space="PSUM") as ps1:
                for d in range(DT):
                    pp = ps1.tile([128, N], F32, tag="pp", name=f"pp{d}",
                                  bufs=2)
                    pq = ps1.tile([128, N], F32, tag="pq", name=f"pq{d}",
                                  bufs=2)
                    for k in range(KT):
                        nc.tensor.matmul(
                            pp,
                            lhsT=w1_sb[d][:, k, :],
                            rhs=reprT_sb[k],
                            start=(k == 0),
                            stop=(k == KT - 1),
                        )
                        nc.tensor.matmul(
                            pq,
                            lhsT=w1_sb[d][:, KT + k, :],
                            rhs=reprT_sb[k],
                            start=(k == 0),
                            stop=(k == KT - 1),
                        )
                    pt = cpool.tile([128, N], BF16, tag=f"pT{d}", name=f"pT{d}")
                    nc.scalar.activation(
                        pt, pp, mybir.ActivationFunctionType.Identity,
                    )
                    qt = cpool.tile([128, N], F32, tag=f"qbT{d}", name=f"qbT{d}")
                    nc.scalar.activation(
                        qt, pq, mybir.ActivationFunctionType.Identity,
                        bias=b1_sb[:, d:d + 1],
                    )
                    pT.append(pt)
                    qbT.append(qt)

            # ---- main loop: 8 mega-tiles of 16 i's ------------------------
            outT_r = outT[:]  # [L, N, N]
            with tc.tile_pool(name="ps2", bufs=2, space="PSUM") as ps2, \
                 tc.tile_pool(name="work", bufs=2) as wpool:
                po_l = [None] * NTILES

                def emit_evict(t):
                    ot = wpool.tile([L, TILE_I * N], F32, tag="ot",
                                    name=f"ot{t}", bufs=2)
                    nc.scalar.copy(ot, po_l[t])
                    po_l[t] = None
                    nc.sync.dma_start(
                        out=outT_r[:, t * TILE_I:(t + 1) * TILE_I, :],
                        in_=ot,
                    )

                slot = 0
                for t in range(NTILES):
                    last = (t == NTILES - 1)
                    # h-gen: 48 per-i ops, d-outer so d=0 ops front-load
                    # while GEMM1 finishes d=1,2.
                    h4 = [[None] * DT for _ in range(TILE_G)]
                    for g in range(TILE_G):
                        for d in range(DT):
                            h4[g][d] = wpool.tile(
                                [128, GROUP * N], BF16, tag=f"h4_{g}_{d}",
                                name=f"h4_{t}_{g}_{d}", bufs=2,
                            )
                    for d in range(DT):
                        for g in range(TILE_G):
                            for il in range(GROUP):
                                i = t * TILE_I + g * GROUP + il
                                dst = h4[g][d][:, il * N:(il + 1) * N]
                                s_prev = (slot * ACT_OPS_PER_TILE) // 48
                                s_next = ((slot + 1) * ACT_OPS_PER_TILE) // 48
                                if s_next != s_prev:
                                    nc.scalar.activation(
                                        dst, pT[d],
                                        mybir.ActivationFunctionType.Relu,
                                        bias=qbT[d][:, i:i + 1],
                                    )
                                else:
                                    nc.vector.tensor_scalar(
                                        dst, pT[d], qbT[d][:, i:i + 1], 0.0,
                                        add, maxop,
                                    )
                                slot = (slot + 1) % 48

                    po = ps2.tile([L, TILE_I * N], F32, tag="po",
                                  name=f"po{t}", bufs=2)
                    po_l[t] = po
                    if not last:
                        # d-outer: stationary W2[d] held across TILE_G MMs;
                        # consecutive MMs write different 512-col slices.
                        for d in range(DT):
                            for g in range(TILE_G):
                                nc.tensor.matmul(
                                    po[:, g * GROUP * N:(g + 1) * GROUP * N],
                                    lhsT=w2_sb[d],
                                    rhs=h4[g][d],
                                    start=(d == 0),
                                    stop=(d == DT - 1),
                                )
                        if t >= 1:
                            emit_evict(t - 1)
                    else:
                        # Last tile: g-outer so each 512-col slice finishes
                        # after 3 MMs and evicts immediately -> short drain.
                        emit_evict(t - 1)
                        for g in range(TILE_G):
                            for d in range(DT):
                                nc.tensor.matmul(
                                    po[:, g * GROUP * N:(g + 1) * GROUP * N],
                                    lhsT=w2_sb[d],
                                    rhs=h4[g][d],
                                    start=(d == 0),
                                    stop=(d == DT - 1),
                                )
                            otg = wpool.tile([L, GROUP * N], F32, tag="otg",
                                             name=f"otg{g}", bufs=2)
                            nc.scalar.copy(
                                otg, po[:, g * GROUP * N:(g + 1) * GROUP * N]
                            )
                            i0 = t * TILE_I + g * GROUP
                            nc.sync.dma_start(
                                out=outT_r[:, i0:i0 + GROUP, :],
                                in_=otg,
                            )
                        po_l[t] = None
    # Bacc defers register allocation + wait legalization to finalize().
    nc.finalize()
    return nc


def kernel(repr_w, W1, b1, W2, b2):
    global LAST_RESULT
    repr_w = np.asarray(repr_w, dtype=np.float32)
    W1 = np.asarray(W1, dtype=np.float32)
    b1 = np.asarray(b1, dtype=np.float32)
    W2 = np.asarray(W2, dtype=np.float32)
    b2 = np.asarray(b2, dtype=np.float32)

    nc = _build_program()

    # w1p[d][p, (half,k)*128+c] = W1[half*H + k*128 + p, d*128 + c]
    w1_r = W1.reshape(2, KT, 128, DT, 128)             # [half,k,p,d,c]
    w1p = np.ascontiguousarray(
        w1_r.transpose(3, 2, 0, 1, 4).reshape(DT, 128, 2 * KT * 128)
    ).astype(ml_dtypes.bfloat16)
    # w2p[p, d*L+l] = W2[d*128+p, l]
    w2p = np.ascontiguousarray(
        W2.reshape(DT, 128, L).transpose(1, 0, 2).reshape(128, DT * L)
    ).astype(ml_dtypes.bfloat16)
    # b1 as per-partition columns: col d = b1[d*128:(d+1)*128]
    b1c = np.ascontiguousarray(b1.reshape(DT, 128).T).astype(np.float32)

    in_maps = []
    for c in range(NCORES):
        # reprP[p, k*N+n] = repr_w[c][n, k*128+p]
        rp = np.ascontiguousarray(
            repr_w[c].T.reshape(KT, 128, N).transpose(1, 0, 2)
            .reshape(128, KT * N)
        ).astype(ml_dtypes.bfloat16)
        in_maps.append({
            "reprP": rp,
            "w1p": w1p,
            "b1c": b1c,
            "w2p": w2p,
        })

    res = run_bass_kernel_spmd(nc, in_maps, core_ids=list(range(NCORES)))
    LAST_RESULT = res

    # outT[l, i, j] -> out[i, j, l]
    out = np.stack(
        [np.moveaxis(res.results[c]["outT"], 0, -1) for c in range(NCORES)],
        axis=0,
    )
    if np.any(b2):
        out = out + b2[None, None, None, :]
    return np.ascontiguousarray(out, dtype=np.float32)


if __name__ == "__main__":
    rng = np.random.default_rng(0)
    inputs = {
        "repr_w": rng.standard_normal((B, N, H), dtype=np.float32),
        "W1": (rng.standard_normal((2 * H, HID)) * 0.02).astype(np.float32),
        "b1": np.zeros(HID, np.float32),
        "W2": (rng.standard_normal((HID, L)) * 0.02).astype(np.float32),
        "b2": np.zeros(L, np.float32),
    }
    outv = kernel(**inputs)
    print("out", outv.shape, outv.dtype, float(np.abs(outv).max()))


# revision 9
# speedup vs baseline: 1.1070x; 1.1070x over previous
"""Trainium2 Bass kernel for nn_BERTCharting (pairwise-concat MLP).

Reference computation (per batch b):
    p = repr_w[b] @ W1[:H]        # [N, HID]
    q = repr_w[b] @ W1[H:]        # [N, HID]
    h[i,j,:] = relu(p[j] + q[i] + b1)
    out[i,j,:] = h[i,j] @ W2 + b2

Sharding: data-parallel over batch B=8 across the 8 NeuronCores (one batch
element per core). No collectives.

v3 design (per core):
  - inputs host-packed so every DMA descriptor is a contiguous >=512B
    per-partition row at full HWDGE rate; load order reprT -> W1[d=0]
    -> W1[d=1] -> W1[d=2] -> W2/b1 so the first GEMM's d=0 chain (and
    then h-gen) starts ~4us in, under the remaining input streams.
  - first GEMM on PE -> pT[d] bf16 / qbT[d]=qT+b1 fp32 (d-sequential;
    p/q accumulation chains interleaved so the PE pipeline overlaps).
  - h-gen: per-i ops [128 dpart, 128 j] (per-partition scalar q[d,i]
    caps the free dim at N=128; DVE dual-op tensor_scalar ~163 ns/op
    pipelined, ACT activation(Relu,bias) ~292 ns/op). 13/48 of ops go
    to ACT, balancing both engines including ACT's eviction load.
  - main GEMM: 8 mega-tiles of 16 i's, PSUM po[100, 2048] (4 banks,
    double-buffered), d-OUTER matmul order (stationary W2[d] held for 4
    MMs into different 512-col slices -> no PSUM output dep between
    neighbours). Last tile g-OUTER with per-group eviction so the
    pipeline drain is ~3us instead of ~9.
  - eviction: ACT copy [100, 2048] fp32 PSUM->SBUF (~1.9us/tile), then
    one DMA with 100 contiguous 8KiB descriptors into outT[l, i, j]
    (host transposes to [i,j,l]).
  - steady state is DVE/ACT-bound on the 6.3M-element relu(p+q) stream
    (both ~100% busy; per-i op floor).
"""

import os
import sys

for _p in ("/opt/trn_rl_repo",):
    if _p not in sys.path and os.path.isdir(_p):
        sys.path.insert(0, _p)

import numpy as np
import ml_dtypes

import concourse.mybir as mybir
from concourse import bacc, bass
from concourse.tile import TileContext
from concourse.bass_utils import run_bass_kernel_spmd


def _ensure_ntff_hook():
    """Provide antenv.axon_hooks (NTFF profile get/set) if the image lacks it,
    and install the ctypes-based profile hook against libaxon_pjrt.so so that
    run_bass_kernel_spmd(trace=True) can capture hardware profiles."""
    try:
        from antenv.axon_hooks import get_axon_ntff_profile_hook  # noqa: F401
        return
    except ImportError:
        pass
    import contextlib
    import ctypes
    import types

    mod = types.ModuleType("antenv.axon_hooks")
    holder = {"hook": None}
    mod.set_axon_ntff_profile_hook = lambda h: holder.__setitem__("hook", h)
    mod.get_axon_ntff_profile_hook = lambda: holder["hook"]
    sys.modules["antenv.axon_hooks"] = mod
    try:
        import antenv
        antenv.axon_hooks = mod
    except ImportError:
        pass

    so_path = "/opt/axon/libaxon_pjrt.so"
    if not os.path.exists(so_path):
        return
    lib = ctypes.CDLL(so_path)
    if not hasattr(lib, "axon_start_nrt_profile"):
        return
    lib.axon_start_nrt_profile.argtypes = [
        ctypes.POINTER(ctypes.c_int64),
        ctypes.c_size_t,
    ]
    lib.axon_start_nrt_profile.restype = ctypes.c_int64
    lib.axon_stop_nrt_profile.argtypes = [ctypes.c_char_p]
    lib.axon_stop_nrt_profile.restype = ctypes.c_int64

    @contextlib.contextmanager
    def _hook(output_dir, device_ids):
        import jax

        jax.devices()
        if device_ids:
            ids = (ctypes.c_int64 * len(device_ids))(*device_ids)
            rc = lib.axon_start_nrt_profile(ids, len(device_ids))
        else:
            rc = lib.axon_start_nrt_profile(None, 0)
        if rc != 0:
            raise RuntimeError(f"axon_start_nrt_profile rc={rc}")
        try:
            yield
        finally:
            n = lib.axon_stop_nrt_profile(str(output_dir).encode())
            print(f"ntff profile: {n} file(s) written to {output_dir}",
                  file=sys.stderr)

    mod.set_axon_ntff_profile_hook(_hook)


_ensure_ntff_hook()

B, N, H = 8, 128, 768
HID, L = 384, 100
NCORES = 8
KT = H // 128          # 6 contraction tiles for the first GEMM
DT = HID // 128        # 3 d-tiles
GROUP = 4              # i's per 512-col psum slice
TILE_G = 4             # groups per psum mega-tile (4 banks)
TILE_I = GROUP * TILE_G        # 16 i's per mega-tile
NTILES = N // TILE_I           # 8 mega-tiles

F32 = mybir.dt.float32
BF16 = mybir.dt.bfloat16

# Of the 48 h-gen ops per mega-tile, this many go to ACT (rest DVE);
# balances DVE (~163ns/op) vs ACT (~292ns/op + 15us eviction load).
ACT_OPS_PER_TILE = 13

# Stash of the last run's BassKernelResults (test harness reads exec_time_ns).
LAST_RESULT = None


def _build_program():
    nc = bacc.Bacc(None, target_bir_lowering=False)

    # Host-packed layouts: per-partition rows contiguous in DRAM.
    reprP = nc.declare_dram_parameter("reprP", [128, KT * N], BF16,
                                      isOutput=False)
    # w1p[d][p, (half,k)*128+c] = W1[half*H + k*128 + p, d*128 + c]
    w1p = nc.declare_dram_parameter("w1p", [DT, 128, 2 * KT * 128], BF16,
                                    isOutput=False)
    # w2p[p, d*L+l] = W2[d*128+p, l]
    w2p = nc.declare_dram_parameter("w2p", [128, DT * L], BF16,
                                    isOutput=False)
    b1c = nc.declare_dram_parameter("b1c", [128, DT], F32, isOutput=False)
    # Output l-major bf16: outT[l, i, j]; host upcasts + transposes to
    # [i, j, l]. bf16 halves the output DMA (~6.5MB -> 3.3MB per core);
    # the added rounding is ~0.4% of scale, well under the 2e-2 budget.
    outT = nc.declare_dram_parameter("outT", [L, N, N], BF16, isOutput=True)

    add = mybir.AluOpType.add
    maxop = mybir.AluOpType.max

    with TileContext(nc) as tc:
        with tc.tile_pool(name="const", bufs=1) as cpool:
            # ---- input loads: one full-rate DMA per chunk ------------------
            # w1[d=0] first so the first GEMM's d=0 chain starts earliest;
            # b1/w2 issued from the idle gpsimd queue to unclog sync.
            w1_sb = []
            w1_tiles = [
                cpool.tile([128, 2 * KT, 128], BF16, tag=f"w1d{d}",
                           name=f"w1d{d}")
                for d in range(DT)
            ]
            nc.sync.dma_start(
                out=w1_tiles[0],
                in_=w1p[0, :, :].rearrange("p (q c) -> p q c", q=2 * KT),
            )
            reprT_big = cpool.tile([128, KT, N], BF16, tag="reprTb",
                                   name="reprTb")
            nc.sync.dma_start(
                out=reprT_big,
                in_=reprP[:].rearrange("p (k n) -> p k n", k=KT),
            )
            reprT_sb = [reprT_big[:, k, :] for k in range(KT)]
            for d in range(1, DT):
                nc.sync.dma_start(
                    out=w1_tiles[d],
                    in_=w1p[d, :, :].rearrange("p (q c) -> p q c", q=2 * KT),
                )
            w1_sb = w1_tiles
            b1_sb = cpool.tile([128, DT], F32, tag="b1c", name="b1sb")
            nc.gpsimd.dma_start(out=b1_sb, in_=b1c[:, :])
            w2_big = cpool.tile([128, DT, L], BF16, tag="w2b", name="w2b")
            nc.gpsimd.dma_start(
                out=w2_big,
                in_=w2p[:].rearrange("p (d l) -> p d l", d=DT),
            )
            w2_sb = [w2_big[:, d, :] for d in range(DT)]

            # ---- first GEMMs: pT, qbT (d-sequential, p/q interleaved) -----
            pT, qbT = [], []
            with tc.tile_pool(name="ps1", bufs=1, space="PSUM") as ps1:
                for d in range(DT):
                    pp = ps1.tile([128, N], F32, tag="pp", name=f"pp{d}",
                                  bufs=2)
                    pq = ps1.tile([128, N], F32, tag="pq", name=f"pq{d}",
                                  bufs=2)
                    for k in range(KT):
                        nc.tensor.matmul(
                            pp,
                            lhsT=w1_sb[d][:, k, :],
                            rhs=reprT_sb[k],
                            start=(k == 0),
                            stop=(k == KT - 1),
                        )
                        nc.tensor.matmul(
                            pq,
                            lhsT=w1_sb[d][:, KT + k, :],
                            rhs=reprT_sb[k],
                            start=(k == 0),
                            stop=(k == KT - 1),
                        )
                    pt = cpool.tile([128, N], BF16, tag=f"pT{d}", name=f"pT{d}")
                    nc.scalar.activation(
                        pt, pp, mybir.ActivationFunctionType.Identity,
                    )
                    qt = cpool.tile([128, N], F32, tag=f"qbT{d}", name=f"qbT{d}")
                    nc.scalar.activation(
                        qt, pq, mybir.ActivationFunctionType.Identity,
                        bias=b1_sb[:, d:d + 1],
                    )
                    pT.append(pt)
                    qbT.append(qt)

            # ---- main loop: 8 mega-tiles of 16 i's ------------------------
            outT_r = outT[:]  # [L, N, N]
            with tc.tile_pool(name="ps2", bufs=2, space="PSUM") as ps2, \
                 tc.tile_pool(name="work", bufs=2) as wpool:
                po_l = [None] * NTILES

                def emit_evict(t):
                    # bf16 staging; DMA issue alternates sync/gpsimd queues
                    # so eviction DMAs overlap in separate DMA queues.
                    ot = wpool.tile([L, TILE_I * N], BF16, tag="ot",
                                    name=f"ot{t}", bufs=4)
                    nc.scalar.copy(ot, po_l[t])
                    po_l[t] = None
                    eng = nc.sync if t % 2 == 0 else nc.gpsimd
                    eng.dma_start(
                        out=outT_r[:, t * TILE_I:(t + 1) * TILE_I, :],
                        in_=ot,
                    )

                slot = 0
                for t in range(NTILES):
                    last = (t == NTILES - 1)
                    # h-gen: 48 per-i ops, d-outer so d=0 ops front-load
                    # while GEMM1 finishes d=1,2.
                    h4 = [[None] * DT for _ in range(TILE_G)]
                    for g in range(TILE_G):
                        for d in range(DT):
                            h4[g][d] = wpool.tile(
                                [128, GROUP * N], BF16, tag=f"h4_{g}_{d}",
                                name=f"h4_{t}_{g}_{d}", bufs=2,
                            )
                    for d in range(DT):
                        for g in range(TILE_G):
                            for il in range(GROUP):
                                i = t * TILE_I + g * GROUP + il
                                dst = h4[g][d][:, il * N:(il + 1) * N]
                                s_prev = (slot * ACT_OPS_PER_TILE) // 48
                                s_next = ((slot + 1) * ACT_OPS_PER_TILE) // 48
                                if s_next != s_prev:
                                    nc.scalar.activation(
                                        dst, pT[d],
                                        mybir.ActivationFunctionType.Relu,
                                        bias=qbT[d][:, i:i + 1],
                                    )
                                else:
                                    nc.vector.tensor_scalar(
                                        dst, pT[d], qbT[d][:, i:i + 1], 0.0,
                                        add, maxop,
                                    )
                                slot = (slot + 1) % 48

                    if not last:
                        po = ps2.tile([L, TILE_I * N], F32, tag="po",
                                      name=f"po{t}", bufs=2)
                        po_l[t] = po
                        # d-outer: stationary W2[d] held across TILE_G MMs;
                        # consecutive MMs write different 512-col slices.
                        for d in range(DT):
                            for g in range(TILE_G):
                                nc.tensor.matmul(
                                    po[:, g * GROUP * N:(g + 1) * GROUP * N],
                                    lhsT=w2_sb[d],
                                    rhs=h4[g][d],
                                    start=(d == 0),
                                    stop=(d == DT - 1),
                                )
                        if t >= 1:
                            emit_evict(t - 1)
                    else:
                        # Last tile: same d-outer MMs, but evict in four
                        # 512-col slices (pipelined with their DMAs across
                        # two queues) to shorten the final drain.
                        po = ps2.tile([L, TILE_I * N], F32, tag="po",
                                      name=f"po{t}", bufs=2)
                        po_l[t] = po
                        emit_evict(t - 1)
                        for d in range(DT):
                            for g in range(TILE_G):
                                nc.tensor.matmul(
                                    po[:, g * GROUP * N:(g + 1) * GROUP * N],
                                    lhsT=w2_sb[d],
                                    rhs=h4[g][d],
                                    start=(d == 0),
                                    stop=(d == DT - 1),
                                )
                        for g in range(TILE_G):
                            otg = wpool.tile([L, GROUP * N], BF16, tag="otg",
                                             name=f"otg{g}", bufs=4)
                            nc.scalar.copy(
                                otg, po[:, g * GROUP * N:(g + 1) * GROUP * N]
                            )
                            i0 = t * TILE_I + g * GROUP
                            eng = nc.sync if g % 2 == 0 else nc.gpsimd
                            eng.dma_start(
                                out=outT_r[:, i0:i0 + GROUP, :],
                                in_=otg,
                            )
                        po_l[t] = None
    # Bacc defers register allocation + wait legalization to finalize().
    nc.finalize()
    return nc


def kernel(repr_w, W1, b1, W2, b2):
    global LAST_RESULT
    repr_w = np.asarray(repr_w, dtype=np.float32)
    W1 = np.asarray(W1, dtype=np.float32)
    b1 = np.asarray(b1, dtype=np.float32)
    W2 = np.asarray(W2, dtype=np.float32)
    b2 = np.asarray(b2, dtype=np.float32)

    nc = _build_program()

    # w1p[d][p, (half,k)*128+c] = W1[half*H + k*128 + p, d*128 + c]
    w1_r = W1.reshape(2, KT, 128, DT, 128)             # [half,k,p,d,c]
    w1p = np.ascontiguousarray(
        w1_r.transpose(3, 2, 0, 1, 4).reshape(DT, 128, 2 * KT * 128)
    ).astype(ml_dtypes.bfloat16)
    # w2p[p, d*L+l] = W2[d*128+p, l]
    w2p = np.ascontiguousarray(
        W2.reshape(DT, 128, L).transpose(1, 0, 2).reshape(128, DT * L)
    ).astype(ml_dtypes.bfloat16)
    # b1 as per-partition columns: col d = b1[d*128:(d+1)*128]
    b1c = np.ascontiguousarray(b1.reshape(DT, 128).T).astype(np.float32)

    in_maps = []
    for c in range(NCORES):
        # reprP[p, k*N+n] = repr_w[c][n, k*128+p]
        rp = np.ascontiguousarray(
            repr_w[c].T.reshape(KT, 128, N).transpose(1, 0, 2)
            .reshape(128, KT * N)
        ).astype(ml_dtypes.bfloat16)
        in_maps.append({
            "reprP": rp,
            "w1p": w1p,
            "b1c": b1c,
            "w2p": w2p,
        })

    res = run_bass_kernel_spmd(nc, in_maps, core_ids=list(range(NCORES)))
    LAST_RESULT = res

    # outT[l, i, j] bf16 -> out[i, j, l] fp32
    out = np.stack(
        [np.moveaxis(res.results[c]["outT"].astype(np.float32), 0, -1)
         for c in range(NCORES)],
        axis=0,
    )
    if np.any(b2):
        out = out + b2[None, None, None, :]
    return np.ascontiguousarray(out, dtype=np.float32)


if __name__ == "__main__":
    rng = np.random.default_rng(0)
    inputs = {
        "repr_w": rng.standard_normal((B, N, H), dtype=np.float32),
        "W1": (rng.standard_normal((2 * H, HID)) * 0.02).astype(np.float32),
        "b1": np.zeros(HID, np.float32),
        "W2": (rng.standard_normal((HID, L)) * 0.02).astype(np.float32),
        "b2": np.zeros(L, np.float32),
    }
    outv = kernel(**inputs)
    print("out", outv.shape, outv.dtype, float(np.abs(outv).max()))
